# revision 27
# baseline (speedup 1.0000x reference)
"""Trainium2 Bass kernel for nn_CameraContrast (proxy-contrastive camera loss).

Strategy (data-parallel over batch, 8 cores):
  Host marshalling (layout only): rows sorted by target id; core c takes 512
  consecutive sorted rows. Per core, proxies are permuted so that the columns
  whose pid matches any of the core's targets land near the start (sort by
  ((pid - t_lo) mod 500, cid)); each row's positive set is then a contiguous
  column range [gs, ge) minus the same-camera subrange [cs, ce).

  Device (per core):
    sims = (f/||f||) @ proxyT / TEMP via fp8-e4m3 DoubleRow matmul. The main
    loop processes proxy columns in groups of 4 chunks (4x512); for each
    (row-tile, kg) stationary weight 4 matmuls run back-to-back into 4 PSUM
    banks. The proxy DMA is software-rotated: a prologue
    loads group 0, and the body prefetches the next iteration's group 0
    during the last group's matmuls, so no DMA sits on the iteration-start
    critical path. ACT evacuates PSUM applying the per-row 1/(||f||*TEMP)
    scale only for the few chunks that can contain positives (per-chunk
    stats: n_pos / sum_pos / sum_exp_pos via fused range-mask passes); for
    all other chunks the per-512-chunk top-8 (DVE max) reads PSUM directly
    at raw-dot scale (the per-row scale is positive, so top-8 order is
    preserved; masked window chunks are offset by -NEGBIG*||f||*TEMP at raw
    scale). Only Copy/Exp run on ACT inside the loop (single activation
    table, no per-iteration table reloads). The device returns the 128
    top-8 candidates per row (bf16, raw scale) plus per-row
    (n_pos, sum_pos, sum_exp_pos, 1/(||f||*TEMP)).

  Host: top-50 of the 128 candidates, rescale, per_row = log(sum exp(top50)
  + pos_exp) - sum_pos/max(n_pos,1) where n_pos>0; loss = sum / B (the
  scalar all-reduce over cores).
"""

import contextlib
from contextlib import ExitStack

import numpy as np
import ml_dtypes

TEMP = 0.07
K = 50
B, D, P = 4096, 2048, 8192
NCORES = 8
BS = B // NCORES          # 512 rows per core
RT = BS // 128            # 4 row-tiles per core
KG = D // 256             # 8 contraction groups (fp8 DoubleRow)
CHUNK = 512
NCHUNK = P // CHUNK       # 16 proxy-column chunks
GRP = 4                   # chunks per group (PSUM banks per row-tile set)
NGRP = NCHUNK // GRP      # 4 chunk groups
NIDS = 500
NEGBIG = 1000.0           # pushes id-matched cols out of the top-k
REPL_IMM = -30000.0       # match_replace fill for extraction rounds

BF16 = ml_dtypes.bfloat16
FP8 = ml_dtypes.float8_e4m3   # == mybir.dt.np(dt.float8e4)


def _prep(features, proxy, targets, cams, pids, cids):
    """Shard + layout marshalling on host. Returns per-core input dicts and the
    per-row-tile positive-chunk windows (shared program structure)."""
    features = np.asarray(features)
    proxy = np.asarray(proxy)
    targets = np.asarray(targets).astype(np.int64)
    cams = np.asarray(cams).astype(np.int64)
    pids = np.asarray(pids).astype(np.int64)
    cids = np.asarray(cids).astype(np.int64)

    order = np.argsort(targets, kind="stable")

    cores = []
    for c in range(NCORES):
        rows = order[c * BS:(c + 1) * BS]
        t = targets[rows]
        cam = cams[rows]
        t_lo = int(t.min())
        spid = (pids - t_lo) % NIDS
        pkey = spid * 8 + cids
        perm = np.argsort(pkey, kind="stable")
        spid_s = spid[perm]
        pkey_s = pkey[perm]
        st = (t - t_lo) % NIDS
        gs = np.searchsorted(spid_s, st, "left")
        ge = np.searchsorted(spid_s, st, "right")
        cs = np.searchsorted(pkey_s, st * 8 + cam, "left")
        ce = np.searchsorted(pkey_s, st * 8 + cam, "right")

        featc = features[rows].astype(np.float32)
        proxyc = proxy[perm].astype(np.float32)
        cores.append(dict(
            in_map={
                "featT": np.ascontiguousarray(featc.T).astype(FP8),
                "feat": np.ascontiguousarray(featc),
                "proxyT": np.ascontiguousarray(proxyc.T).astype(FP8),
                "rowmeta": np.ascontiguousarray(
                    np.stack([gs, ge, cs, ce], axis=1).astype(np.float32)),
            },
            gs=gs, ge=ge,
        ))

    windows = []
    for r in range(RT):
        lo, hi = P, 0
        for c in cores:
            sl = slice(r * 128, r * 128 + 128)
            g0, g1 = c["gs"][sl], c["ge"][sl]
            ne = g1 > g0
            if ne.any():
                lo = min(lo, int(g0[ne].min()))
                hi = max(hi, int(g1[ne].max()))
        windows.append(
            [] if lo >= hi else list(range(lo // CHUNK, (hi - 1) // CHUNK + 1)))

    # idx input carries only the chunks any window needs (global column ids)
    wchunks = sorted({pc for w in windows for pc in w})
    idx_cols = np.concatenate(
        [np.arange(pc * CHUNK, (pc + 1) * CHUNK, dtype=np.int16)
         for pc in wchunks]) if wchunks else np.zeros(CHUNK, np.int16)
    idx_row = np.ascontiguousarray(
        np.broadcast_to(idx_cols, (128, len(idx_cols))))
    for c in cores:
        c["in_map"]["idx"] = idx_row

    in_maps = [c["in_map"] for c in cores]
    return in_maps, windows


def _finalize(results):
    """Host epilogue: per-row top-50 over the device's 128 per-chunk-top-8
    candidates, exp-sum, log, mean subtraction, and the scalar all-reduce."""
    total = 0.0
    for c in range(NCORES):
        o = np.asarray(results[c]["out"], dtype=np.float64).reshape(128, RT, 4)
        npos, spos, pexp, rn = o[..., 0], o[..., 1], o[..., 2], o[..., 3]
        u = np.asarray(results[c]["uout"]).reshape(128, RT, NCHUNK * 8)
        u = u.astype(np.float64)
        top = -np.partition(-u, K - 1, axis=-1)[..., :K]
        top = top * rn[..., None]
        S = np.exp(top).sum(axis=-1) + pexp
        mean = spos / np.maximum(npos, 1.0)
        per = np.where(npos > 0, np.log(np.maximum(S, 1e-300)) - mean, 0.0)
        total += per.sum()
    return np.array([total / B], dtype=np.float32)


def _build_program(windows, stage="full", loop_n=None, rhs_dmas=8):
    import concourse.bacc as bacc
    import concourse.mybir as mybir
    from concourse.tile import TileContext

    dt = mybir.dt
    Alu = mybir.AluOpType
    Act = mybir.ActivationFunctionType
    mmdt = dt.float8e4

    wchunks = sorted({pc for w in windows for pc in w})
    wpos = {pc: i for i, pc in enumerate(wchunks)}
    nwc = max(1, len(wchunks))

    nc = bacc.Bacc("TRN2", num_devices=NCORES)

    featT_d = nc.dram_tensor("featT", [D, BS], mmdt, kind="ExternalInput")
    feat_d = nc.dram_tensor("feat", [BS, D], dt.float32, kind="ExternalInput")
    proxyT_d = nc.dram_tensor("proxyT", [D, P], mmdt, kind="ExternalInput")
    idx_d = nc.dram_tensor("idx", [128, nwc * CHUNK], dt.int16,
                           kind="ExternalInput")
    meta_d = nc.dram_tensor("rowmeta", [BS, 4], dt.float32, kind="ExternalInput")
    out_d = nc.dram_tensor("out", [128, RT * 4], dt.float32,
                           kind="ExternalOutput")
    u_d = nc.dram_tensor("uout", [128, RT * NCHUNK * 8], dt.bfloat16,
                         kind="ExternalOutput")

    def dma_group(rhs_tile, g):
        # one DMA per (kg, i) slice of proxyT columns [g*GRP*CHUNK, ...)
        for kg in range(KG):
            for i in range(2):
                nc.sync.dma_start(
                    out=rhs_tile[:, kg, i, :],
                    in_=proxyT_d.ap()[
                        kg * 256 + i * 128:kg * 256 + (i + 1) * 128,
                        g * GRP * CHUNK:(g + 1) * GRP * CHUNK])

    with TileContext(nc) as tc, ExitStack() as ctx:
        const = ctx.enter_context(tc.tile_pool(name="const", bufs=1))
        psum = ctx.enter_context(tc.tile_pool(name="psum", bufs=8, space="PSUM"))
        evp = ctx.enter_context(tc.tile_pool(name="ev", bufs=8))
        mk = ctx.enter_context(tc.tile_pool(name="mk", bufs=3))
        keep = ctx.enter_context(tc.tile_pool(name="keep", bufs=1))
        fin = ctx.enter_context(tc.tile_pool(name="fin", bufs=2))

        # ---- resident loads ------------------------------------------------
        featT_s = const.tile([128, KG, 2, BS], mmdt)
        for kg in range(KG):
            nc.sync.dma_start(
                out=featT_s[:, kg, :, :],
                in_=featT_d.ap()[kg * 256:(kg + 1) * 256, :]
                .rearrange("(i p) b -> p i b", p=128))
        idx_s = const.tile([128, nwc * CHUNK], dt.int16)
        nc.gpsimd.dma_start(out=idx_s, in_=idx_d.ap())
        meta_s = const.tile([128, RT, 4], dt.float32)
        nc.gpsimd.dma_start(
            out=meta_s, in_=meta_d.ap().rearrange("(rt p) m -> p rt m", p=128))

        # rotating proxy buffers (explicit, so the body can prefetch next
        # iteration's group 0 while the tail computes)
        rhsA = keep.tile([128, KG, 2, GRP * CHUNK], mmdt, tag="rhsA",
                         name="rhsA")
        rhsB = keep.tile([128, KG, 2, GRP * CHUNK], mmdt, tag="rhsB",
                         name="rhsB")
        rbuf = [rhsA, rhsB]
        dma_group(rhsA, 0)   # prologue: group 0 for the first iteration

        # ---- per-row 1/(||f||*TEMP) ----------------------------------------
        rn_s = keep.tile([128, RT], dt.float32, tag="rn")
        nOr_tmp = keep.tile([128, RT], dt.float32, tag="nOrt", name="nOr_tmp")
        for r in range(RT):
            ft = mk.tile([128, D], dt.float32, tag="ftile", bufs=1, name="ft")
            nc.gpsimd.dma_start(
                out=ft, in_=feat_d.ap()[r * 128:(r + 1) * 128, :])
            sq = mk.tile([128, D], dt.bfloat16, tag="sqscr", bufs=1, name="sq")
            n2 = fin.tile([128, 1], dt.float32, tag="n2")
            nc.vector.scalar_tensor_tensor(
                sq, in0=ft, scalar=0.0, in1=ft, op0=Alu.bypass,
                op1=Alu.mult, accum_out=n2)
            sn = fin.tile([128, 1], dt.float32, tag="sn")
            nc.scalar.activation(out=sn, in_=n2, func=Act.Sqrt, scale=TEMP * TEMP)
            nc.vector.reciprocal(out=rn_s[:, r:r + 1], in_=sn)
            nc.vector.tensor_scalar(nOr_tmp[:, r:r + 1], sn, -NEGBIG, None,
                                    op0=Alu.mult)
        # raw-scale mask constant: -NEGBIG * ||f|| * TEMP per row (applying
        # it to the un-normalized dot products keeps masked columns out of
        # the top-8 at raw scale)
        nOr_s = nOr_tmp
        # dummy Exp so the preamble exits with the exp/copy table loaded and
        # the loop body needs no per-iteration LoadActFuncSet
        dume = fin.tile([1, RT], dt.float32, tag="dume", name="dume")
        nc.scalar.activation(out=dume, in_=rn_s[0:1, :], func=Act.Exp)

        # ---- persistent per-row-tile state ---------------------------------
        U_all = keep.tile([128, RT, NCHUNK * 8], dt.bfloat16, tag="U",
                          name="U_all")
        U_s = [U_all[:, r, :] for r in range(RT)]
        NW = [max(1, len(windows[r])) for r in range(RT)]
        np_s = [keep.tile([128, NW[r]], dt.float32, tag=f"npos{r}", name=f"npos{r}")
                for r in range(RT)]
        sp_s = [keep.tile([128, NW[r]], dt.float32, tag=f"spos{r}", name=f"spos{r}")
                for r in range(RT)]
        pe_s = [keep.tile([128, NW[r]], dt.float32, tag=f"pexp{r}", name=f"pexp{r}")
                for r in range(RT)]
        for r in range(RT):
            nc.vector.memset(np_s[r], 0.0)
            nc.vector.memset(sp_s[r], 0.0)
            nc.vector.memset(pe_s[r], 0.0)
        stats_s = keep.tile([128, RT * 4], dt.float32, tag="stats",
                            name="stats")
        # rn for the host (un-scaling the shipped raw candidates)
        for r in range(RT):
            nc.vector.tensor_copy(out=stats_s[:, 4 * r + 3:4 * r + 4],
                                  in_=rn_s[:, r:r + 1])

        # ---- precompute window masks (DVE idle at kernel start) ------------
        wm = {}
        for r in range(RT):
            for w, pc in enumerate(windows[r]):
                if stage not in ("full", "noepi"):
                    continue
                wp = wpos[pc]
                ich = idx_s[:, wp * CHUNK:(wp + 1) * CHUNK]
                gsc = meta_s[:, r, 0:1]
                gec = meta_s[:, r, 1:2]
                csc = meta_s[:, r, 2:3]
                cec = meta_s[:, r, 3:4]
                a = mk.tile([128, CHUNK], dt.bfloat16, tag="a")
                nc.vector.tensor_scalar(a, ich, gsc, None, op0=Alu.is_ge)
                m1 = keep.tile([128, CHUNK], dt.bfloat16, tag=f"m1_{r}_{w}",
                               name=f"m1_{r}_{w}")
                nc.vector.scalar_tensor_tensor(
                    m1, in0=ich, scalar=gec, in1=a, op0=Alu.is_lt, op1=Alu.mult)
                bt = mk.tile([128, CHUNK], dt.bfloat16, tag="b")
                nc.vector.tensor_scalar(bt, ich, csc, None, op0=Alu.is_ge)
                m2 = mk.tile([128, CHUNK], dt.bfloat16, tag="m2")
                nc.vector.scalar_tensor_tensor(
                    m2, in0=ich, scalar=cec, in1=bt, op0=Alu.is_lt, op1=Alu.mult)
                mp = keep.tile([128, CHUNK], dt.bfloat16, tag=f"mp_{r}_{w}",
                               name=f"mp_{r}_{w}")
                nc.vector.scalar_tensor_tensor(
                    mp, in0=m2, scalar=-1.0, in1=m1, op0=Alu.mult, op1=Alu.add,
                    accum_out=np_s[r][:, w:w + 1])
                wm[(r, pc)] = (m1, mp)

        def stats_r(r):
            """Reduce the window accumulators for row-tile r into stats_s
            (npos, spos, pexp); top-50 selection + log run on host."""
            nc.vector.tensor_reduce(out=stats_s[:, 4 * r:4 * r + 1],
                                    in_=np_s[r],
                                    axis=mybir.AxisListType.X, op=Alu.add)
            nc.vector.tensor_reduce(out=stats_s[:, 4 * r + 1:4 * r + 2],
                                    in_=sp_s[r],
                                    axis=mybir.AxisListType.X, op=Alu.add)
            nc.vector.tensor_reduce(out=stats_s[:, 4 * r + 2:4 * r + 3],
                                    in_=pe_s[r],
                                    axis=mybir.AxisListType.X, op=Alu.add)

        # last group index containing any window chunk, per row-tile
        gw_last = [max((pc // GRP for pc in windows[r]), default=0)
                   for r in range(RT)]

        loop_cm = tc.For_i(0, loop_n, 1) if loop_n else contextlib.nullcontext()
        with loop_cm:
            # ---- main loop: 4-chunk groups, weight reused across 4 banks ---
            # Each (g, r, kg) block of 4 same-weight matmuls is pinned to an
            # increasing scheduler timestamp so the tile scheduler (whose PE
            # cost model runs DoubleRow ~2.3x too fast and would otherwise
            # chase freed PSUM banks down kg-chains) keeps the blocks whole;
            # the dedupe pass then drops 3 of every 4 LDWEIGHTS.
            for g in range(NGRP):
                rhs = rbuf[g % 2]
                # prefetch: next group into the other buffer; on the last
                # group, refill THIS iteration's group-0 buffer for the next
                # iteration (same DRAM contents every iteration)
                nxt = g + 1 if g + 1 < NGRP else 0
                dma_group(rbuf[nxt % 2], nxt)
                for r in range(RT):
                    pss = [psum.tile([128, CHUNK], dt.float32, tag="ps",
                                     name=f"ps{c}")
                           for c in range(GRP)]
                    for kg in range(KG):
                        for c in range(GRP):
                            nc.tensor.matmul(
                                pss[c],
                                lhsT=featT_s[:, kg, :, r * 128:(r + 1) * 128],
                                rhs=rhs[:, kg, :, c * CHUNK:(c + 1) * CHUNK],
                                start=(kg == 0),
                                stop=(kg == KG - 1),
                                perf_mode=mybir.MatmulPerfMode.DoubleRow,
                            )
                    if stage == "mmonly":
                        continue
                    for c in range(GRP):
                        pc = g * GRP + c
                        if pc in windows[r] and stage in ("full", "noepi"):
                            w = windows[r].index(pc)
                            m1, mp = wm[(r, pc)]
                            sims = evp.tile([128, CHUNK], dt.bfloat16,
                                            tag="sims")
                            nc.scalar.activation(
                                out=sims, in_=pss[c], func=Act.Copy,
                                scale=rn_s[:, r:r + 1])
                            scr = mk.tile([128, CHUNK], dt.bfloat16, tag="scr")
                            nc.vector.scalar_tensor_tensor(
                                scr, in0=sims, scalar=0.0, in1=mp,
                                op0=Alu.bypass, op1=Alu.mult,
                                accum_out=sp_s[r][:, w:w + 1])
                            ec = evp.tile([128, CHUNK], dt.bfloat16, tag="ec")
                            nc.scalar.activation(out=ec, in_=sims, func=Act.Exp)
                            scr2 = mk.tile([128, CHUNK], dt.bfloat16, tag="scr2")
                            nc.vector.scalar_tensor_tensor(
                                scr2, in0=ec, scalar=0.0, in1=mp,
                                op0=Alu.bypass, op1=Alu.mult,
                                accum_out=pe_s[r][:, w:w + 1])
                            # masked candidates at raw-dot scale for max8
                            negv = mk.tile([128, CHUNK], dt.bfloat16, tag="negv")
                            nc.vector.scalar_tensor_tensor(
                                negv, in0=m1, scalar=nOr_s[:, r:r + 1],
                                in1=pss[c], op0=Alu.mult, op1=Alu.add)
                            maxsrc = negv
                        else:
                            maxsrc = pss[c]
                        nc.vector.max(out=U_s[r][:, pc * 8:(pc + 1) * 8],
                                      in_=maxsrc)
                    if g == gw_last[r] and stage == "full":
                        stats_r(r)
                    if stage == "full":
                        if g == NGRP - 2:
                            # ship chunks [0, 3*GRP) once groups 0-2 are done
                            nc.sync.dma_start(
                                out=u_d.ap()[:, r * NCHUNK * 8:
                                             r * NCHUNK * 8 + (NGRP - 1) * GRP * 8],
                                in_=U_s[r][:, :(NGRP - 1) * GRP * 8])
                        elif g == NGRP - 1:
                            # last group: one small DMA per chunk, so the tail
                            # only waits on the final chunk's max8
                            for c in range(GRP):
                                pc = g * GRP + c
                                nc.sync.dma_start(
                                    out=u_d.ap()[:, r * NCHUNK * 8 + pc * 8:
                                                 r * NCHUNK * 8 + (pc + 1) * 8],
                                    in_=U_s[r][:, pc * 8:(pc + 1) * 8])
                if g == max(gw_last) and stage == "full":
                    nc.sync.dma_start(out=out_d.ap(), in_=stats_s)

            if stage == "full":
                pass  # stats DMA emitted inside the group loop
            else:
                nc.vector.memset(stats_s[:, 0:1], 0.0)
                nc.sync.dma_start(
                    out=out_d.ap()[0:1, 0:1], in_=stats_s[0:1, 0:1])
                nc.sync.dma_start(
                    out=u_d.ap()[0:1, 0:1], in_=U_all[0:1, 0, 0:1])

    nc.compile()
    return nc


_PROGRAM_CACHE = {}


def _get_program(windows):
    key = tuple(tuple(w) for w in windows)
    if key not in _PROGRAM_CACHE:
        _PROGRAM_CACHE[key] = _build_program(windows)
    return _PROGRAM_CACHE[key]


def kernel(features, proxy, targets, cams, pids, cids):
    from concourse.bass_utils import run_bass_kernel_spmd

    in_maps, windows = _prep(features, proxy, targets, cams, pids, cids)
    nc = _get_program(windows)
    res = run_bass_kernel_spmd(nc, in_maps, core_ids=list(range(NCORES)))
    return _finalize(res.results)


# revision 28
# speedup vs baseline: 1.0073x; 1.0073x over previous
"""Trainium2 Bass kernel for nn_CameraContrast (proxy-contrastive camera loss).

Strategy (data-parallel over batch, 8 cores):
  Host marshalling (layout only): rows sorted by target id; core c takes 512
  consecutive sorted rows. Per core, proxies are permuted so that the columns
  whose pid matches any of the core's targets land near the start (sort by
  ((pid - t_lo) mod 500, cid)); each row's positive set is then a contiguous
  column range [gs, ge) minus the same-camera subrange [cs, ce).

  Device (per core):
    sims = (f/||f||) @ proxyT / TEMP via fp8-e4m3 DoubleRow matmul. The main
    loop processes proxy columns in groups of 4 chunks (4x512); for each
    (row-tile, kg) stationary weight 4 matmuls run back-to-back into 4 PSUM
    banks. The proxy DMA is software-rotated: a prologue
    loads group 0, and the body prefetches the next iteration's group 0
    during the last group's matmuls, so no DMA sits on the iteration-start
    critical path. ACT evacuates PSUM applying the per-row 1/(||f||*TEMP)
    scale only for the few chunks that can contain positives (per-chunk
    stats: n_pos / sum_pos / sum_exp_pos via fused range-mask passes); for
    all other chunks the per-512-chunk top-8 (DVE max) reads PSUM directly
    at raw-dot scale (the per-row scale is positive, so top-8 order is
    preserved; masked window chunks are offset by -NEGBIG*||f||*TEMP at raw
    scale). Only Copy/Exp run on ACT inside the loop (single activation
    table, no per-iteration table reloads). The device returns the 128
    top-8 candidates per row (bf16, raw scale) plus per-row
    (n_pos, sum_pos, sum_exp_pos, 1/(||f||*TEMP)).

  Host: top-50 of the 128 candidates, rescale, per_row = log(sum exp(top50)
  + pos_exp) - sum_pos/max(n_pos,1) where n_pos>0; loss = sum / B (the
  scalar all-reduce over cores).
"""

import contextlib
from contextlib import ExitStack

import numpy as np
import ml_dtypes

TEMP = 0.07
K = 50
B, D, P = 4096, 2048, 8192
NCORES = 8
BS = B // NCORES          # 512 rows per core
RT = BS // 128            # 4 row-tiles per core
KG = D // 256             # 8 contraction groups (fp8 DoubleRow)
CHUNK = 512
NCHUNK = P // CHUNK       # 16 proxy-column chunks
GRP = 4                   # chunks per group (PSUM banks per row-tile set)
NGRP = NCHUNK // GRP      # 4 chunk groups
NIDS = 500
NEGBIG = 1000.0           # pushes id-matched cols out of the top-k
REPL_IMM = -30000.0       # match_replace fill for extraction rounds

BF16 = ml_dtypes.bfloat16
FP8 = ml_dtypes.float8_e4m3   # == mybir.dt.np(dt.float8e4)


def _prep(features, proxy, targets, cams, pids, cids):
    """Shard + layout marshalling on host. Returns per-core input dicts and the
    per-row-tile positive-chunk windows (shared program structure)."""
    features = np.asarray(features)
    proxy = np.asarray(proxy)
    targets = np.asarray(targets).astype(np.int64)
    cams = np.asarray(cams).astype(np.int64)
    pids = np.asarray(pids).astype(np.int64)
    cids = np.asarray(cids).astype(np.int64)

    order = np.argsort(targets, kind="stable")

    cores = []
    for c in range(NCORES):
        rows = order[c * BS:(c + 1) * BS]
        t = targets[rows]
        cam = cams[rows]
        t_lo = int(t.min())
        spid = (pids - t_lo) % NIDS
        pkey = spid * 8 + cids
        perm = np.argsort(pkey, kind="stable")
        spid_s = spid[perm]
        pkey_s = pkey[perm]
        st = (t - t_lo) % NIDS
        gs = np.searchsorted(spid_s, st, "left")
        ge = np.searchsorted(spid_s, st, "right")
        cs = np.searchsorted(pkey_s, st * 8 + cam, "left")
        ce = np.searchsorted(pkey_s, st * 8 + cam, "right")

        featc = features[rows].astype(np.float32)
        proxyc = proxy[perm].astype(np.float32)
        cores.append(dict(
            in_map={
                "featT": np.ascontiguousarray(featc.T).astype(FP8),
                "feat": np.ascontiguousarray(featc),
                "proxyT": np.ascontiguousarray(proxyc.T).astype(FP8),
                "rowmeta": np.ascontiguousarray(
                    np.stack([gs, ge, cs, ce], axis=1).astype(np.float32)),
            },
            gs=gs, ge=ge,
        ))

    windows = []
    for r in range(RT):
        lo, hi = P, 0
        for c in cores:
            sl = slice(r * 128, r * 128 + 128)
            g0, g1 = c["gs"][sl], c["ge"][sl]
            ne = g1 > g0
            if ne.any():
                lo = min(lo, int(g0[ne].min()))
                hi = max(hi, int(g1[ne].max()))
        windows.append(
            [] if lo >= hi else list(range(lo // CHUNK, (hi - 1) // CHUNK + 1)))

    # idx input carries only the chunks any window needs (global column ids)
    wchunks = sorted({pc for w in windows for pc in w})
    idx_cols = np.concatenate(
        [np.arange(pc * CHUNK, (pc + 1) * CHUNK, dtype=np.int16)
         for pc in wchunks]) if wchunks else np.zeros(CHUNK, np.int16)
    idx_row = np.ascontiguousarray(
        np.broadcast_to(idx_cols, (128, len(idx_cols))))
    for c in cores:
        c["in_map"]["idx"] = idx_row

    in_maps = [c["in_map"] for c in cores]
    return in_maps, windows


def _finalize(results):
    """Host epilogue: per-row top-50 over the device's 128 per-chunk-top-8
    candidates, exp-sum, log, mean subtraction, and the scalar all-reduce."""
    total = 0.0
    for c in range(NCORES):
        o = np.asarray(results[c]["out"], dtype=np.float64).reshape(128, RT, 4)
        npos, spos, pexp, rn = o[..., 0], o[..., 1], o[..., 2], o[..., 3]
        u = np.asarray(results[c]["uout"]).reshape(128, RT, NCHUNK * 8)
        u = u.astype(np.float64)
        top = -np.partition(-u, K - 1, axis=-1)[..., :K]
        top = top * rn[..., None]
        S = np.exp(top).sum(axis=-1) + pexp
        mean = spos / np.maximum(npos, 1.0)
        per = np.where(npos > 0, np.log(np.maximum(S, 1e-300)) - mean, 0.0)
        total += per.sum()
    return np.array([total / B], dtype=np.float32)


def _build_program(windows, stage="full", loop_n=None, rhs_dmas=8):
    import concourse.bacc as bacc
    import concourse.mybir as mybir
    from concourse.tile import TileContext

    dt = mybir.dt
    Alu = mybir.AluOpType
    Act = mybir.ActivationFunctionType
    mmdt = dt.float8e4

    wchunks = sorted({pc for w in windows for pc in w})
    wpos = {pc: i for i, pc in enumerate(wchunks)}
    nwc = max(1, len(wchunks))

    nc = bacc.Bacc("TRN2", num_devices=NCORES)

    featT_d = nc.dram_tensor("featT", [D, BS], mmdt, kind="ExternalInput")
    feat_d = nc.dram_tensor("feat", [BS, D], dt.float32, kind="ExternalInput")
    proxyT_d = nc.dram_tensor("proxyT", [D, P], mmdt, kind="ExternalInput")
    idx_d = nc.dram_tensor("idx", [128, nwc * CHUNK], dt.int16,
                           kind="ExternalInput")
    meta_d = nc.dram_tensor("rowmeta", [BS, 4], dt.float32, kind="ExternalInput")
    out_d = nc.dram_tensor("out", [128, RT * 4], dt.float32,
                           kind="ExternalOutput")
    u_d = nc.dram_tensor("uout", [128, RT * NCHUNK * 8], dt.bfloat16,
                         kind="ExternalOutput")

    def dma_group(rhs_tile, g):
        # one DMA per (kg, i) slice of proxyT columns [g*GRP*CHUNK, ...)
        for kg in range(KG):
            for i in range(2):
                nc.sync.dma_start(
                    out=rhs_tile[:, kg, i, :],
                    in_=proxyT_d.ap()[
                        kg * 256 + i * 128:kg * 256 + (i + 1) * 128,
                        g * GRP * CHUNK:(g + 1) * GRP * CHUNK])

    with TileContext(nc) as tc, ExitStack() as ctx:
        const = ctx.enter_context(tc.tile_pool(name="const", bufs=1))
        psum = ctx.enter_context(tc.tile_pool(name="psum", bufs=8, space="PSUM"))
        evp = ctx.enter_context(tc.tile_pool(name="ev", bufs=8))
        mk = ctx.enter_context(tc.tile_pool(name="mk", bufs=3))
        keep = ctx.enter_context(tc.tile_pool(name="keep", bufs=1))
        fin = ctx.enter_context(tc.tile_pool(name="fin", bufs=2))

        # ---- resident loads ------------------------------------------------
        featT_s = const.tile([128, KG, 2, BS], mmdt)
        for kg in range(KG):
            nc.sync.dma_start(
                out=featT_s[:, kg, :, :],
                in_=featT_d.ap()[kg * 256:(kg + 1) * 256, :]
                .rearrange("(i p) b -> p i b", p=128))
        idx_s = const.tile([128, nwc * CHUNK], dt.int16)
        nc.gpsimd.dma_start(out=idx_s, in_=idx_d.ap())
        meta_s = const.tile([128, RT, 4], dt.float32)
        nc.gpsimd.dma_start(
            out=meta_s, in_=meta_d.ap().rearrange("(rt p) m -> p rt m", p=128))

        # rotating proxy buffers (explicit, so the body can prefetch next
        # iteration's group 0 while the tail computes)
        rhsA = keep.tile([128, KG, 2, GRP * CHUNK], mmdt, tag="rhsA",
                         name="rhsA")
        rhsB = keep.tile([128, KG, 2, GRP * CHUNK], mmdt, tag="rhsB",
                         name="rhsB")
        rbuf = [rhsA, rhsB]
        dma_group(rhsA, 0)   # prologue: group 0 for the first iteration

        # ---- per-row 1/(||f||*TEMP) ----------------------------------------
        rn_s = keep.tile([128, RT], dt.float32, tag="rn")
        nOr_tmp = keep.tile([128, RT], dt.float32, tag="nOrt", name="nOr_tmp")
        for r in range(RT):
            ft = mk.tile([128, D], dt.float32, tag="ftile", bufs=1, name="ft")
            nc.gpsimd.dma_start(
                out=ft, in_=feat_d.ap()[r * 128:(r + 1) * 128, :])
            sq = mk.tile([128, D], dt.bfloat16, tag="sqscr", bufs=1, name="sq")
            n2 = fin.tile([128, 1], dt.float32, tag="n2")
            nc.vector.scalar_tensor_tensor(
                sq, in0=ft, scalar=0.0, in1=ft, op0=Alu.bypass,
                op1=Alu.mult, accum_out=n2)
            sn = fin.tile([128, 1], dt.float32, tag="sn")
            nc.scalar.activation(out=sn, in_=n2, func=Act.Sqrt, scale=TEMP * TEMP)
            nc.vector.reciprocal(out=rn_s[:, r:r + 1], in_=sn)
            nc.vector.tensor_scalar(nOr_tmp[:, r:r + 1], sn, -NEGBIG, None,
                                    op0=Alu.mult)
        # raw-scale mask constant: -NEGBIG * ||f|| * TEMP per row (applying
        # it to the un-normalized dot products keeps masked columns out of
        # the top-8 at raw scale)
        nOr_s = nOr_tmp
        # dummy Exp so the preamble exits with the exp/copy table loaded and
        # the loop body needs no per-iteration LoadActFuncSet
        dume = fin.tile([1, RT], dt.float32, tag="dume", name="dume")
        nc.scalar.activation(out=dume, in_=rn_s[0:1, :], func=Act.Exp)

        # ---- persistent per-row-tile state ---------------------------------
        U_all = keep.tile([128, RT, NCHUNK * 8], dt.bfloat16, tag="U",
                          name="U_all")
        U_s = [U_all[:, r, :] for r in range(RT)]
        NW = [max(1, len(windows[r])) for r in range(RT)]
        np_s = [keep.tile([128, NW[r]], dt.float32, tag=f"npos{r}", name=f"npos{r}")
                for r in range(RT)]
        sp_s = [keep.tile([128, NW[r]], dt.float32, tag=f"spos{r}", name=f"spos{r}")
                for r in range(RT)]
        pe_s = [keep.tile([128, NW[r]], dt.float32, tag=f"pexp{r}", name=f"pexp{r}")
                for r in range(RT)]
        for r in range(RT):
            nc.vector.memset(np_s[r], 0.0)
            nc.vector.memset(sp_s[r], 0.0)
            nc.vector.memset(pe_s[r], 0.0)
        stats_s = keep.tile([128, RT * 4], dt.float32, tag="stats",
                            name="stats")
        # rn for the host (un-scaling the shipped raw candidates)
        for r in range(RT):
            nc.vector.tensor_copy(out=stats_s[:, 4 * r + 3:4 * r + 4],
                                  in_=rn_s[:, r:r + 1])

        # ---- precompute window masks (DVE idle at kernel start) ------------
        wm = {}
        for r in range(RT):
            for w, pc in enumerate(windows[r]):
                if stage not in ("full", "noepi"):
                    continue
                wp = wpos[pc]
                ich = idx_s[:, wp * CHUNK:(wp + 1) * CHUNK]
                gsc = meta_s[:, r, 0:1]
                gec = meta_s[:, r, 1:2]
                csc = meta_s[:, r, 2:3]
                cec = meta_s[:, r, 3:4]
                a = mk.tile([128, CHUNK], dt.bfloat16, tag="a")
                nc.vector.tensor_scalar(a, ich, gsc, None, op0=Alu.is_ge)
                m1 = keep.tile([128, CHUNK], dt.bfloat16, tag=f"m1_{r}_{w}",
                               name=f"m1_{r}_{w}")
                nc.vector.scalar_tensor_tensor(
                    m1, in0=ich, scalar=gec, in1=a, op0=Alu.is_lt, op1=Alu.mult)
                bt = mk.tile([128, CHUNK], dt.bfloat16, tag="b")
                nc.vector.tensor_scalar(bt, ich, csc, None, op0=Alu.is_ge)
                m2 = mk.tile([128, CHUNK], dt.bfloat16, tag="m2")
                nc.vector.scalar_tensor_tensor(
                    m2, in0=ich, scalar=cec, in1=bt, op0=Alu.is_lt, op1=Alu.mult)
                mp = keep.tile([128, CHUNK], dt.bfloat16, tag=f"mp_{r}_{w}",
                               name=f"mp_{r}_{w}")
                nc.vector.scalar_tensor_tensor(
                    mp, in0=m2, scalar=-1.0, in1=m1, op0=Alu.mult, op1=Alu.add,
                    accum_out=np_s[r][:, w:w + 1])
                wm[(r, pc)] = (m1, mp)

        def stats_r(r):
            """Reduce the window accumulators for row-tile r into stats_s
            (npos, spos, pexp); top-50 selection + log run on host."""
            nc.vector.tensor_reduce(out=stats_s[:, 4 * r:4 * r + 1],
                                    in_=np_s[r],
                                    axis=mybir.AxisListType.X, op=Alu.add)
            nc.vector.tensor_reduce(out=stats_s[:, 4 * r + 1:4 * r + 2],
                                    in_=sp_s[r],
                                    axis=mybir.AxisListType.X, op=Alu.add)
            nc.vector.tensor_reduce(out=stats_s[:, 4 * r + 2:4 * r + 3],
                                    in_=pe_s[r],
                                    axis=mybir.AxisListType.X, op=Alu.add)

        # last group index containing any window chunk, per row-tile
        gw_last = [max((pc // GRP for pc in windows[r]), default=0)
                   for r in range(RT)]

        loop_cm = tc.For_i(0, loop_n, 1) if loop_n else contextlib.nullcontext()
        with loop_cm:
            # ---- main loop: 4-chunk groups, weight reused across 4 banks ---
            # Each (g, r, kg) block of 4 same-weight matmuls is pinned to an
            # increasing scheduler timestamp so the tile scheduler (whose PE
            # cost model runs DoubleRow ~2.3x too fast and would otherwise
            # chase freed PSUM banks down kg-chains) keeps the blocks whole;
            # the dedupe pass then drops 3 of every 4 LDWEIGHTS.
            for g in range(NGRP):
                rhs = rbuf[g % 2]
                # prefetch: next group into the other buffer; on the last
                # group, refill THIS iteration's group-0 buffer for the next
                # iteration (same DRAM contents every iteration)
                nxt = g + 1 if g + 1 < NGRP else 0
                dma_group(rbuf[nxt % 2], nxt)
                for r in range(RT):
                    pss = [psum.tile([128, CHUNK], dt.float32, tag="ps",
                                     name=f"ps{c}")
                           for c in range(GRP)]
                    for kg in range(KG):
                        for c in range(GRP):
                            nc.tensor.matmul(
                                pss[c],
                                lhsT=featT_s[:, kg, :, r * 128:(r + 1) * 128],
                                rhs=rhs[:, kg, :, c * CHUNK:(c + 1) * CHUNK],
                                start=(kg == 0),
                                stop=(kg == KG - 1),
                                perf_mode=mybir.MatmulPerfMode.DoubleRow,
                            )
                    if stage == "mmonly":
                        continue
                    for c in range(GRP):
                        pc = g * GRP + c
                        if pc in windows[r] and stage in ("full", "noepi"):
                            w = windows[r].index(pc)
                            m1, mp = wm[(r, pc)]
                            sims = evp.tile([128, CHUNK], dt.bfloat16,
                                            tag="sims")
                            nc.scalar.activation(
                                out=sims, in_=pss[c], func=Act.Copy,
                                scale=rn_s[:, r:r + 1])
                            scr = mk.tile([128, CHUNK], dt.bfloat16, tag="scr")
                            nc.vector.scalar_tensor_tensor(
                                scr, in0=sims, scalar=0.0, in1=mp,
                                op0=Alu.bypass, op1=Alu.mult,
                                accum_out=sp_s[r][:, w:w + 1])
                            ec = evp.tile([128, CHUNK], dt.bfloat16, tag="ec")
                            nc.scalar.activation(out=ec, in_=sims, func=Act.Exp)
                            scr2 = mk.tile([128, CHUNK], dt.bfloat16, tag="scr2")
                            nc.vector.scalar_tensor_tensor(
                                scr2, in0=ec, scalar=0.0, in1=mp,
                                op0=Alu.bypass, op1=Alu.mult,
                                accum_out=pe_s[r][:, w:w + 1])
                            # masked candidates at raw-dot scale for max8
                            negv = mk.tile([128, CHUNK], dt.bfloat16, tag="negv")
                            nc.vector.scalar_tensor_tensor(
                                negv, in0=m1, scalar=nOr_s[:, r:r + 1],
                                in1=pss[c], op0=Alu.mult, op1=Alu.add)
                            maxsrc = negv
                        else:
                            maxsrc = pss[c]
                        nc.vector.max(out=U_s[r][:, pc * 8:(pc + 1) * 8],
                                      in_=maxsrc)
                    if g == gw_last[r] and stage == "full":
                        stats_r(r)
                    if stage == "full":
                        if g == NGRP - 2:
                            # ship chunks [0, 3*GRP) once groups 0-2 are done
                            nc.sync.dma_start(
                                out=u_d.ap()[:, r * NCHUNK * 8:
                                             r * NCHUNK * 8 + (NGRP - 1) * GRP * 8],
                                in_=U_s[r][:, :(NGRP - 1) * GRP * 8])
                        elif g == NGRP - 1:
                            # ship the last group's candidates in one DMA
                            nc.sync.dma_start(
                                out=u_d.ap()[:, r * NCHUNK * 8 + (NGRP - 1) * GRP * 8:
                                             (r + 1) * NCHUNK * 8],
                                in_=U_s[r][:, (NGRP - 1) * GRP * 8:])
                if g == max(gw_last) and stage == "full":
                    nc.sync.dma_start(out=out_d.ap(), in_=stats_s)

            if stage == "full":
                pass  # stats DMA emitted inside the group loop
            else:
                nc.vector.memset(stats_s[:, 0:1], 0.0)
                nc.sync.dma_start(
                    out=out_d.ap()[0:1, 0:1], in_=stats_s[0:1, 0:1])
                nc.sync.dma_start(
                    out=u_d.ap()[0:1, 0:1], in_=U_all[0:1, 0, 0:1])

    nc.compile()
    return nc


_PROGRAM_CACHE = {}


def _get_program(windows):
    key = tuple(tuple(w) for w in windows)
    if key not in _PROGRAM_CACHE:
        _PROGRAM_CACHE[key] = _build_program(windows)
    return _PROGRAM_CACHE[key]


def kernel(features, proxy, targets, cams, pids, cids):
    from concourse.bass_utils import run_bass_kernel_spmd

    in_maps, windows = _prep(features, proxy, targets, cams, pids, cids)
    nc = _get_program(windows)
    res = run_bass_kernel_spmd(nc, in_maps, core_ids=list(range(NCORES)))
    return _finalize(res.results)


# revision 29
# speedup vs baseline: 1.0092x; 1.0019x over previous
"""Trainium2 Bass kernel for nn_CameraContrast (proxy-contrastive camera loss).

Strategy (data-parallel over batch, 8 cores):
  Host marshalling (layout only): rows sorted by target id; core c takes 512
  consecutive sorted rows. Per core, proxies are permuted so that the columns
  whose pid matches any of the core's targets land near the start (sort by
  ((pid - t_lo) mod 500, cid)); each row's positive set is then a contiguous
  column range [gs, ge) minus the same-camera subrange [cs, ce).

  Device (per core):
    sims = (f/||f||) @ proxyT / TEMP via fp8-e4m3 DoubleRow matmul. The main
    loop processes proxy columns in groups of 4 chunks (4x512); for each
    (row-tile, kg) stationary weight 4 matmuls run back-to-back into 4 PSUM
    banks. The proxy DMA is software-rotated: a prologue
    loads group 0, and the body prefetches the next iteration's group 0
    during the last group's matmuls, so no DMA sits on the iteration-start
    critical path. ACT evacuates PSUM applying the per-row 1/(||f||*TEMP)
    scale only for the few chunks that can contain positives (per-chunk
    stats: n_pos / sum_pos / sum_exp_pos via fused range-mask passes); for
    all other chunks the per-512-chunk top-8 (DVE max) reads PSUM directly
    at raw-dot scale (the per-row scale is positive, so top-8 order is
    preserved; masked window chunks are offset by -NEGBIG*||f||*TEMP at raw
    scale). Only Copy/Exp run on ACT inside the loop (single activation
    table, no per-iteration table reloads). The device returns the 128
    top-8 candidates per row (bf16, raw scale) plus per-row
    (n_pos, sum_pos, sum_exp_pos, 1/(||f||*TEMP)).

  Host: top-50 of the 128 candidates, rescale, per_row = log(sum exp(top50)
  + pos_exp) - sum_pos/max(n_pos,1) where n_pos>0; loss = sum / B (the
  scalar all-reduce over cores).
"""

import contextlib
from contextlib import ExitStack

import numpy as np
import ml_dtypes

TEMP = 0.07
K = 50
B, D, P = 4096, 2048, 8192
NCORES = 8
BS = B // NCORES          # 512 rows per core
RT = BS // 128            # 4 row-tiles per core
KG = D // 256             # 8 contraction groups (fp8 DoubleRow)
CHUNK = 512
NCHUNK = P // CHUNK       # 16 proxy-column chunks
GRP = 4                   # chunks per group (PSUM banks per row-tile set)
NGRP = NCHUNK // GRP      # 4 chunk groups
NIDS = 500
NEGBIG = 1000.0           # pushes id-matched cols out of the top-k
REPL_IMM = -30000.0       # match_replace fill for extraction rounds

BF16 = ml_dtypes.bfloat16
FP8 = ml_dtypes.float8_e4m3   # == mybir.dt.np(dt.float8e4)


def _prep(features, proxy, targets, cams, pids, cids):
    """Shard + layout marshalling on host. Returns per-core input dicts and the
    per-row-tile positive-chunk windows (shared program structure)."""
    features = np.asarray(features)
    proxy = np.asarray(proxy)
    targets = np.asarray(targets).astype(np.int64)
    cams = np.asarray(cams).astype(np.int64)
    pids = np.asarray(pids).astype(np.int64)
    cids = np.asarray(cids).astype(np.int64)

    order = np.argsort(targets, kind="stable")

    cores = []
    for c in range(NCORES):
        rows = order[c * BS:(c + 1) * BS]
        t = targets[rows]
        cam = cams[rows]
        t_lo = int(t.min())
        spid = (pids - t_lo) % NIDS
        pkey = spid * 8 + cids
        perm = np.argsort(pkey, kind="stable")
        spid_s = spid[perm]
        pkey_s = pkey[perm]
        st = (t - t_lo) % NIDS
        gs = np.searchsorted(spid_s, st, "left")
        ge = np.searchsorted(spid_s, st, "right")
        cs = np.searchsorted(pkey_s, st * 8 + cam, "left")
        ce = np.searchsorted(pkey_s, st * 8 + cam, "right")

        featc = features[rows].astype(np.float32)
        proxyc = proxy[perm].astype(np.float32)
        cores.append(dict(
            in_map={
                "featT": np.ascontiguousarray(featc.T).astype(FP8),
                "feat": np.ascontiguousarray(featc),
                "proxyT": np.ascontiguousarray(proxyc.T).astype(FP8),
                "rowmeta": np.ascontiguousarray(
                    np.stack([gs, ge, cs, ce], axis=1).astype(np.float32)),
            },
            gs=gs, ge=ge,
        ))

    windows = []
    for r in range(RT):
        lo, hi = P, 0
        for c in cores:
            sl = slice(r * 128, r * 128 + 128)
            g0, g1 = c["gs"][sl], c["ge"][sl]
            ne = g1 > g0
            if ne.any():
                lo = min(lo, int(g0[ne].min()))
                hi = max(hi, int(g1[ne].max()))
        windows.append(
            [] if lo >= hi else list(range(lo // CHUNK, (hi - 1) // CHUNK + 1)))

    # idx input carries only the chunks any window needs (global column ids)
    wchunks = sorted({pc for w in windows for pc in w})
    idx_cols = np.concatenate(
        [np.arange(pc * CHUNK, (pc + 1) * CHUNK, dtype=np.int16)
         for pc in wchunks]) if wchunks else np.zeros(CHUNK, np.int16)
    idx_row = np.ascontiguousarray(
        np.broadcast_to(idx_cols, (128, len(idx_cols))))
    for c in cores:
        c["in_map"]["idx"] = idx_row

    in_maps = [c["in_map"] for c in cores]
    return in_maps, windows


def _finalize(results):
    """Host epilogue: per-row top-50 over the device's 128 per-chunk-top-8
    candidates, exp-sum, log, mean subtraction, and the scalar all-reduce."""
    total = 0.0
    for c in range(NCORES):
        o = np.asarray(results[c]["out"], dtype=np.float64).reshape(128, RT, 4)
        npos, spos, pexp, rn = o[..., 0], o[..., 1], o[..., 2], o[..., 3]
        u = np.asarray(results[c]["uout"]).reshape(128, RT, NCHUNK * 8)
        u = u.astype(np.float64)
        top = -np.partition(-u, K - 1, axis=-1)[..., :K]
        top = top * rn[..., None]
        S = np.exp(top).sum(axis=-1) + pexp
        mean = spos / np.maximum(npos, 1.0)
        per = np.where(npos > 0, np.log(np.maximum(S, 1e-300)) - mean, 0.0)
        total += per.sum()
    return np.array([total / B], dtype=np.float32)


def _build_program(windows, stage="full", loop_n=None, rhs_dmas=8):
    import concourse.bacc as bacc
    import concourse.mybir as mybir
    from concourse.tile import TileContext

    dt = mybir.dt
    Alu = mybir.AluOpType
    Act = mybir.ActivationFunctionType
    mmdt = dt.float8e4

    wchunks = sorted({pc for w in windows for pc in w})
    wpos = {pc: i for i, pc in enumerate(wchunks)}
    nwc = max(1, len(wchunks))

    nc = bacc.Bacc("TRN2", num_devices=NCORES)

    featT_d = nc.dram_tensor("featT", [D, BS], mmdt, kind="ExternalInput")
    feat_d = nc.dram_tensor("feat", [BS, D], dt.float32, kind="ExternalInput")
    proxyT_d = nc.dram_tensor("proxyT", [D, P], mmdt, kind="ExternalInput")
    idx_d = nc.dram_tensor("idx", [128, nwc * CHUNK], dt.int16,
                           kind="ExternalInput")
    meta_d = nc.dram_tensor("rowmeta", [BS, 4], dt.float32, kind="ExternalInput")
    out_d = nc.dram_tensor("out", [128, RT * 4], dt.float32,
                           kind="ExternalOutput")
    u_d = nc.dram_tensor("uout", [128, RT * NCHUNK * 8], dt.bfloat16,
                         kind="ExternalOutput")

    def dma_group(rhs_tile, g):
        # one DMA per (kg, i) slice of proxyT columns [g*GRP*CHUNK, ...)
        for kg in range(KG):
            for i in range(2):
                nc.sync.dma_start(
                    out=rhs_tile[:, kg, i, :],
                    in_=proxyT_d.ap()[
                        kg * 256 + i * 128:kg * 256 + (i + 1) * 128,
                        g * GRP * CHUNK:(g + 1) * GRP * CHUNK])

    with TileContext(nc) as tc, ExitStack() as ctx:
        const = ctx.enter_context(tc.tile_pool(name="const", bufs=1))
        psum = ctx.enter_context(tc.tile_pool(name="psum", bufs=8, space="PSUM"))
        evp = ctx.enter_context(tc.tile_pool(name="ev", bufs=8))
        mk = ctx.enter_context(tc.tile_pool(name="mk", bufs=3))
        keep = ctx.enter_context(tc.tile_pool(name="keep", bufs=1))
        fin = ctx.enter_context(tc.tile_pool(name="fin", bufs=2))

        # ---- resident loads ------------------------------------------------
        featT_s = const.tile([128, KG, 2, BS], mmdt)
        for kg in range(KG):
            nc.sync.dma_start(
                out=featT_s[:, kg, :, :],
                in_=featT_d.ap()[kg * 256:(kg + 1) * 256, :]
                .rearrange("(i p) b -> p i b", p=128))
        idx_s = const.tile([128, nwc * CHUNK], dt.int16)
        nc.gpsimd.dma_start(out=idx_s, in_=idx_d.ap())
        meta_s = const.tile([128, RT, 4], dt.float32)
        nc.gpsimd.dma_start(
            out=meta_s, in_=meta_d.ap().rearrange("(rt p) m -> p rt m", p=128))

        # rotating proxy buffers (explicit, so the body can prefetch next
        # iteration's group 0 while the tail computes)
        rhsA = keep.tile([128, KG, 2, GRP * CHUNK], mmdt, tag="rhsA",
                         name="rhsA")
        rhsB = keep.tile([128, KG, 2, GRP * CHUNK], mmdt, tag="rhsB",
                         name="rhsB")
        rbuf = [rhsA, rhsB]
        dma_group(rhsA, 0)   # prologue: group 0 for the first iteration

        # ---- per-row 1/(||f||*TEMP) ----------------------------------------
        rn_s = keep.tile([128, RT], dt.float32, tag="rn")
        nOr_tmp = keep.tile([128, RT], dt.float32, tag="nOrt", name="nOr_tmp")
        for r in range(RT):
            ft = mk.tile([128, D], dt.float32, tag="ftile", bufs=1, name="ft")
            nc.gpsimd.dma_start(
                out=ft, in_=feat_d.ap()[r * 128:(r + 1) * 128, :])
            sq = mk.tile([128, D], dt.bfloat16, tag="sqscr", bufs=1, name="sq")
            n2 = fin.tile([128, 1], dt.float32, tag="n2")
            nc.vector.scalar_tensor_tensor(
                sq, in0=ft, scalar=0.0, in1=ft, op0=Alu.bypass,
                op1=Alu.mult, accum_out=n2)
            sn = fin.tile([128, 1], dt.float32, tag="sn")
            nc.scalar.activation(out=sn, in_=n2, func=Act.Sqrt, scale=TEMP * TEMP)
            nc.vector.reciprocal(out=rn_s[:, r:r + 1], in_=sn)
            nc.vector.tensor_scalar(nOr_tmp[:, r:r + 1], sn, -NEGBIG, None,
                                    op0=Alu.mult)
        # raw-scale mask constant: -NEGBIG * ||f|| * TEMP per row (applying
        # it to the un-normalized dot products keeps masked columns out of
        # the top-8 at raw scale)
        nOr_s = nOr_tmp
        # dummy Exp so the preamble exits with the exp/copy table loaded and
        # the loop body needs no per-iteration LoadActFuncSet
        dume = fin.tile([1, RT], dt.float32, tag="dume", name="dume")
        nc.scalar.activation(out=dume, in_=rn_s[0:1, :], func=Act.Exp)

        # ---- persistent per-row-tile state ---------------------------------
        U_all = keep.tile([128, RT, NCHUNK * 8], dt.bfloat16, tag="U",
                          name="U_all")
        U_s = [U_all[:, r, :] for r in range(RT)]
        NW = [max(1, len(windows[r])) for r in range(RT)]
        np_s = [keep.tile([128, NW[r]], dt.float32, tag=f"npos{r}", name=f"npos{r}")
                for r in range(RT)]
        sp_s = [keep.tile([128, NW[r]], dt.float32, tag=f"spos{r}", name=f"spos{r}")
                for r in range(RT)]
        pe_s = [keep.tile([128, NW[r]], dt.float32, tag=f"pexp{r}", name=f"pexp{r}")
                for r in range(RT)]
        for r in range(RT):
            nc.vector.memset(np_s[r], 0.0)
            nc.vector.memset(sp_s[r], 0.0)
            nc.vector.memset(pe_s[r], 0.0)
        stats_s = keep.tile([128, RT * 4], dt.float32, tag="stats",
                            name="stats")
        # rn for the host (un-scaling the shipped raw candidates)
        for r in range(RT):
            nc.vector.tensor_copy(out=stats_s[:, 4 * r + 3:4 * r + 4],
                                  in_=rn_s[:, r:r + 1])

        # ---- precompute window masks (DVE idle at kernel start) ------------
        wm = {}
        for r in range(RT):
            for w, pc in enumerate(windows[r]):
                if stage not in ("full", "noepi"):
                    continue
                wp = wpos[pc]
                ich = idx_s[:, wp * CHUNK:(wp + 1) * CHUNK]
                gsc = meta_s[:, r, 0:1]
                gec = meta_s[:, r, 1:2]
                csc = meta_s[:, r, 2:3]
                cec = meta_s[:, r, 3:4]
                a = mk.tile([128, CHUNK], dt.bfloat16, tag="a")
                nc.vector.tensor_scalar(a, ich, gsc, None, op0=Alu.is_ge)
                m1 = keep.tile([128, CHUNK], dt.bfloat16, tag=f"m1_{r}_{w}",
                               name=f"m1_{r}_{w}")
                nc.vector.scalar_tensor_tensor(
                    m1, in0=ich, scalar=gec, in1=a, op0=Alu.is_lt, op1=Alu.mult)
                bt = mk.tile([128, CHUNK], dt.bfloat16, tag="b")
                nc.vector.tensor_scalar(bt, ich, csc, None, op0=Alu.is_ge)
                m2 = mk.tile([128, CHUNK], dt.bfloat16, tag="m2")
                nc.vector.scalar_tensor_tensor(
                    m2, in0=ich, scalar=cec, in1=bt, op0=Alu.is_lt, op1=Alu.mult)
                mp = keep.tile([128, CHUNK], dt.bfloat16, tag=f"mp_{r}_{w}",
                               name=f"mp_{r}_{w}")
                nc.vector.scalar_tensor_tensor(
                    mp, in0=m2, scalar=-1.0, in1=m1, op0=Alu.mult, op1=Alu.add,
                    accum_out=np_s[r][:, w:w + 1])
                wm[(r, pc)] = (m1, mp)

        def stats_r(r):
            """Reduce the window accumulators for row-tile r into stats_s
            (npos, spos, pexp); top-50 selection + log run on host."""
            nc.vector.tensor_reduce(out=stats_s[:, 4 * r:4 * r + 1],
                                    in_=np_s[r],
                                    axis=mybir.AxisListType.X, op=Alu.add)
            nc.vector.tensor_reduce(out=stats_s[:, 4 * r + 1:4 * r + 2],
                                    in_=sp_s[r],
                                    axis=mybir.AxisListType.X, op=Alu.add)
            nc.vector.tensor_reduce(out=stats_s[:, 4 * r + 2:4 * r + 3],
                                    in_=pe_s[r],
                                    axis=mybir.AxisListType.X, op=Alu.add)

        # last group index containing any window chunk, per row-tile
        gw_last = [max((pc // GRP for pc in windows[r]), default=0)
                   for r in range(RT)]

        loop_cm = tc.For_i(0, loop_n, 1) if loop_n else contextlib.nullcontext()
        with loop_cm:
            # ---- main loop: 4-chunk groups, weight reused across 4 banks ---
            # Each (g, r, kg) block of 4 same-weight matmuls is pinned to an
            # increasing scheduler timestamp so the tile scheduler (whose PE
            # cost model runs DoubleRow ~2.3x too fast and would otherwise
            # chase freed PSUM banks down kg-chains) keeps the blocks whole;
            # the dedupe pass then drops 3 of every 4 LDWEIGHTS.
            for g in range(NGRP):
                rhs = rbuf[g % 2]
                # prefetch: next group into the other buffer; on the last
                # group, refill THIS iteration's group-0 buffer for the next
                # iteration (same DRAM contents every iteration)
                nxt = g + 1 if g + 1 < NGRP else 0
                dma_group(rbuf[nxt % 2], nxt)
                for r in range(RT):
                    pss = [psum.tile([128, CHUNK], dt.float32, tag="ps",
                                     name=f"ps{c}")
                           for c in range(GRP)]
                    # the very last section runs as two 2-bank halves so the
                    # first half's top-8s overlap the second half's matmuls,
                    # shortening the iteration tail
                    last_sec = (g == NGRP - 1 and r == RT - 1)
                    halves = ([range(0, GRP // 2), range(GRP // 2, GRP)]
                              if last_sec else [range(GRP)])
                    for cs in halves:
                        for kg in range(KG):
                            for c in cs:
                                nc.tensor.matmul(
                                    pss[c],
                                    lhsT=featT_s[:, kg, :,
                                                 r * 128:(r + 1) * 128],
                                    rhs=rhs[:, kg, :,
                                            c * CHUNK:(c + 1) * CHUNK],
                                    start=(kg == 0),
                                    stop=(kg == KG - 1),
                                    perf_mode=mybir.MatmulPerfMode.DoubleRow,
                                )
                    if stage == "mmonly":
                        continue
                    for c in range(GRP):
                        pc = g * GRP + c
                        if pc in windows[r] and stage in ("full", "noepi"):
                            w = windows[r].index(pc)
                            m1, mp = wm[(r, pc)]
                            sims = evp.tile([128, CHUNK], dt.bfloat16,
                                            tag="sims")
                            nc.scalar.activation(
                                out=sims, in_=pss[c], func=Act.Copy,
                                scale=rn_s[:, r:r + 1])
                            scr = mk.tile([128, CHUNK], dt.bfloat16, tag="scr")
                            nc.vector.scalar_tensor_tensor(
                                scr, in0=sims, scalar=0.0, in1=mp,
                                op0=Alu.bypass, op1=Alu.mult,
                                accum_out=sp_s[r][:, w:w + 1])
                            ec = evp.tile([128, CHUNK], dt.bfloat16, tag="ec")
                            nc.scalar.activation(out=ec, in_=sims, func=Act.Exp)
                            scr2 = mk.tile([128, CHUNK], dt.bfloat16, tag="scr2")
                            nc.vector.scalar_tensor_tensor(
                                scr2, in0=ec, scalar=0.0, in1=mp,
                                op0=Alu.bypass, op1=Alu.mult,
                                accum_out=pe_s[r][:, w:w + 1])
                            # masked candidates at raw-dot scale for max8
                            negv = mk.tile([128, CHUNK], dt.bfloat16, tag="negv")
                            nc.vector.scalar_tensor_tensor(
                                negv, in0=m1, scalar=nOr_s[:, r:r + 1],
                                in1=pss[c], op0=Alu.mult, op1=Alu.add)
                            maxsrc = negv
                        else:
                            maxsrc = pss[c]
                        nc.vector.max(out=U_s[r][:, pc * 8:(pc + 1) * 8],
                                      in_=maxsrc)
                    if g == gw_last[r] and stage == "full":
                        stats_r(r)
                    if stage == "full":
                        if g == NGRP - 2:
                            # ship chunks [0, 3*GRP) once groups 0-2 are done
                            nc.sync.dma_start(
                                out=u_d.ap()[:, r * NCHUNK * 8:
                                             r * NCHUNK * 8 + (NGRP - 1) * GRP * 8],
                                in_=U_s[r][:, :(NGRP - 1) * GRP * 8])
                        elif g == NGRP - 1:
                            # ship the last group's candidates in one DMA
                            nc.sync.dma_start(
                                out=u_d.ap()[:, r * NCHUNK * 8 + (NGRP - 1) * GRP * 8:
                                             (r + 1) * NCHUNK * 8],
                                in_=U_s[r][:, (NGRP - 1) * GRP * 8:])
                if g == max(gw_last) and stage == "full":
                    nc.sync.dma_start(out=out_d.ap(), in_=stats_s)

            if stage == "full":
                pass  # stats DMA emitted inside the group loop
            else:
                nc.vector.memset(stats_s[:, 0:1], 0.0)
                nc.sync.dma_start(
                    out=out_d.ap()[0:1, 0:1], in_=stats_s[0:1, 0:1])
                nc.sync.dma_start(
                    out=u_d.ap()[0:1, 0:1], in_=U_all[0:1, 0, 0:1])

    nc.compile()
    return nc


_PROGRAM_CACHE = {}


def _get_program(windows):
    key = tuple(tuple(w) for w in windows)
    if key not in _PROGRAM_CACHE:
        _PROGRAM_CACHE[key] = _build_program(windows)
    return _PROGRAM_CACHE[key]


def kernel(features, proxy, targets, cams, pids, cids):
    from concourse.bass_utils import run_bass_kernel_spmd

    in_maps, windows = _prep(features, proxy, targets, cams, pids, cids)
    nc = _get_program(windows)
    res = run_bass_kernel_spmd(nc, in_maps, core_ids=list(range(NCORES)))
    return _finalize(res.results)


# revision 30
# speedup vs baseline: 1.0126x; 1.0033x over previous
"""Trainium2 Bass kernel for nn_CameraContrast (proxy-contrastive camera loss).

Strategy (data-parallel over batch, 8 cores):
  Host marshalling (layout only): rows sorted by target id; core c takes 512
  consecutive sorted rows. Per core, proxies are permuted so that the columns
  whose pid matches any of the core's targets land near the start (sort by
  ((pid - t_lo) mod 500, cid)); each row's positive set is then a contiguous
  column range [gs, ge) minus the same-camera subrange [cs, ce).

  Device (per core):
    sims = (f/||f||) @ proxyT / TEMP via fp8-e4m3 DoubleRow matmul. The main
    loop processes proxy columns in groups of 4 chunks (4x512); for each
    (row-tile, kg) stationary weight 4 matmuls run back-to-back into 4 PSUM
    banks. The proxy DMA is software-rotated: a prologue
    loads group 0, and the body prefetches the next iteration's group 0
    during the last group's matmuls, so no DMA sits on the iteration-start
    critical path. ACT evacuates PSUM applying the per-row 1/(||f||*TEMP)
    scale only for the few chunks that can contain positives (per-chunk
    stats: n_pos / sum_pos / sum_exp_pos via fused range-mask passes); for
    all other chunks the per-512-chunk top-8 (DVE max) reads PSUM directly
    at raw-dot scale (the per-row scale is positive, so top-8 order is
    preserved; masked window chunks are offset by -NEGBIG*||f||*TEMP at raw
    scale). Only Copy/Exp run on ACT inside the loop (single activation
    table, no per-iteration table reloads). The device returns the 128
    top-8 candidates per row (bf16, raw scale) plus per-row
    (n_pos, sum_pos, sum_exp_pos, 1/(||f||*TEMP)).

  Host: top-50 of the 128 candidates, rescale, per_row = log(sum exp(top50)
  + pos_exp) - sum_pos/max(n_pos,1) where n_pos>0; loss = sum / B (the
  scalar all-reduce over cores).
"""

import contextlib
from contextlib import ExitStack

import numpy as np
import ml_dtypes

TEMP = 0.07
K = 50
B, D, P = 4096, 2048, 8192
NCORES = 8
BS = B // NCORES          # 512 rows per core
RT = BS // 128            # 4 row-tiles per core
KG = D // 256             # 8 contraction groups (fp8 DoubleRow)
CHUNK = 512
NCHUNK = P // CHUNK       # 16 proxy-column chunks
GRP = 4                   # chunks per group (PSUM banks per row-tile set)
NGRP = NCHUNK // GRP      # 4 chunk groups
NIDS = 500
NEGBIG = 1000.0           # pushes id-matched cols out of the top-k
REPL_IMM = -30000.0       # match_replace fill for extraction rounds

BF16 = ml_dtypes.bfloat16
FP8 = ml_dtypes.float8_e4m3   # == mybir.dt.np(dt.float8e4)


def _prep(features, proxy, targets, cams, pids, cids):
    """Shard + layout marshalling on host. Returns per-core input dicts and the
    per-row-tile positive-chunk windows (shared program structure)."""
    features = np.asarray(features)
    proxy = np.asarray(proxy)
    targets = np.asarray(targets).astype(np.int64)
    cams = np.asarray(cams).astype(np.int64)
    pids = np.asarray(pids).astype(np.int64)
    cids = np.asarray(cids).astype(np.int64)

    order = np.argsort(targets, kind="stable")

    cores = []
    for c in range(NCORES):
        rows = order[c * BS:(c + 1) * BS]
        t = targets[rows]
        cam = cams[rows]
        t_lo = int(t.min())
        spid = (pids - t_lo) % NIDS
        pkey = spid * 8 + cids
        perm = np.argsort(pkey, kind="stable")
        spid_s = spid[perm]
        pkey_s = pkey[perm]
        st = (t - t_lo) % NIDS
        gs = np.searchsorted(spid_s, st, "left")
        ge = np.searchsorted(spid_s, st, "right")
        cs = np.searchsorted(pkey_s, st * 8 + cam, "left")
        ce = np.searchsorted(pkey_s, st * 8 + cam, "right")

        featc = features[rows].astype(np.float32)
        proxyc = proxy[perm].astype(np.float32)
        cores.append(dict(
            in_map={
                "featT": np.ascontiguousarray(featc.T).astype(FP8),
                "feat": np.ascontiguousarray(featc),
                "proxyT": np.ascontiguousarray(proxyc.T).astype(FP8),
                "rowmeta": np.ascontiguousarray(
                    np.stack([gs, ge, cs, ce], axis=1).astype(np.float32)),
            },
            gs=gs, ge=ge,
        ))

    windows = []
    for r in range(RT):
        lo, hi = P, 0
        for c in cores:
            sl = slice(r * 128, r * 128 + 128)
            g0, g1 = c["gs"][sl], c["ge"][sl]
            ne = g1 > g0
            if ne.any():
                lo = min(lo, int(g0[ne].min()))
                hi = max(hi, int(g1[ne].max()))
        windows.append(
            [] if lo >= hi else list(range(lo // CHUNK, (hi - 1) // CHUNK + 1)))

    # idx input carries only the chunks any window needs (global column ids)
    wchunks = sorted({pc for w in windows for pc in w})
    idx_cols = np.concatenate(
        [np.arange(pc * CHUNK, (pc + 1) * CHUNK, dtype=np.int16)
         for pc in wchunks]) if wchunks else np.zeros(CHUNK, np.int16)
    idx_row = np.ascontiguousarray(
        np.broadcast_to(idx_cols, (128, len(idx_cols))))
    for c in cores:
        c["in_map"]["idx"] = idx_row

    in_maps = [c["in_map"] for c in cores]
    return in_maps, windows


def _finalize(results):
    """Host epilogue: per-row top-50 over the device's 128 per-chunk-top-8
    candidates, exp-sum, log, mean subtraction, and the scalar all-reduce."""
    total = 0.0
    for c in range(NCORES):
        o = np.asarray(results[c]["out"], dtype=np.float64).reshape(128, RT, 4)
        npos, spos, pexp, rn = o[..., 0], o[..., 1], o[..., 2], o[..., 3]
        u = np.asarray(results[c]["uout"]).reshape(128, RT, NCHUNK * 8)
        u = u.astype(np.float64)
        top = -np.partition(-u, K - 1, axis=-1)[..., :K]
        top = top * rn[..., None]
        S = np.exp(top).sum(axis=-1) + pexp
        mean = spos / np.maximum(npos, 1.0)
        per = np.where(npos > 0, np.log(np.maximum(S, 1e-300)) - mean, 0.0)
        total += per.sum()
    return np.array([total / B], dtype=np.float32)


def _build_program(windows, stage="full", loop_n=None, rhs_dmas=8):
    import concourse.bacc as bacc
    import concourse.mybir as mybir
    from concourse.tile import TileContext

    dt = mybir.dt
    Alu = mybir.AluOpType
    Act = mybir.ActivationFunctionType
    mmdt = dt.float8e4

    wchunks = sorted({pc for w in windows for pc in w})
    wpos = {pc: i for i, pc in enumerate(wchunks)}
    nwc = max(1, len(wchunks))

    nc = bacc.Bacc("TRN2", num_devices=NCORES)

    featT_d = nc.dram_tensor("featT", [D, BS], mmdt, kind="ExternalInput")
    feat_d = nc.dram_tensor("feat", [BS, D], dt.float32, kind="ExternalInput")
    proxyT_d = nc.dram_tensor("proxyT", [D, P], mmdt, kind="ExternalInput")
    idx_d = nc.dram_tensor("idx", [128, nwc * CHUNK], dt.int16,
                           kind="ExternalInput")
    meta_d = nc.dram_tensor("rowmeta", [BS, 4], dt.float32, kind="ExternalInput")
    out_d = nc.dram_tensor("out", [128, RT * 4], dt.float32,
                           kind="ExternalOutput")
    u_d = nc.dram_tensor("uout", [128, RT * NCHUNK * 8], dt.bfloat16,
                         kind="ExternalOutput")

    def dma_group(rhs_tile, g):
        # one DMA per (kg, i) slice of proxyT columns [g*GRP*CHUNK, ...)
        for kg in range(KG):
            for i in range(2):
                nc.sync.dma_start(
                    out=rhs_tile[:, kg, i, :],
                    in_=proxyT_d.ap()[
                        kg * 256 + i * 128:kg * 256 + (i + 1) * 128,
                        g * GRP * CHUNK:(g + 1) * GRP * CHUNK])

    with TileContext(nc) as tc, ExitStack() as ctx:
        const = ctx.enter_context(tc.tile_pool(name="const", bufs=1))
        psum = ctx.enter_context(tc.tile_pool(name="psum", bufs=8, space="PSUM"))
        evp = ctx.enter_context(tc.tile_pool(name="ev", bufs=8))
        mk = ctx.enter_context(tc.tile_pool(name="mk", bufs=3))
        keep = ctx.enter_context(tc.tile_pool(name="keep", bufs=1))
        fin = ctx.enter_context(tc.tile_pool(name="fin", bufs=2))

        # ---- resident loads ------------------------------------------------
        featT_s = const.tile([128, KG, 2, BS], mmdt)
        for kg in range(KG):
            nc.sync.dma_start(
                out=featT_s[:, kg, :, :],
                in_=featT_d.ap()[kg * 256:(kg + 1) * 256, :]
                .rearrange("(i p) b -> p i b", p=128))
        idx_s = const.tile([128, nwc * CHUNK], dt.int16)
        nc.gpsimd.dma_start(out=idx_s, in_=idx_d.ap())
        meta_s = const.tile([128, RT, 4], dt.float32)
        nc.gpsimd.dma_start(
            out=meta_s, in_=meta_d.ap().rearrange("(rt p) m -> p rt m", p=128))

        # rotating proxy buffers (explicit, so the body can prefetch next
        # iteration's group 0 while the tail computes)
        rhsA = keep.tile([128, KG, 2, GRP * CHUNK], mmdt, tag="rhsA",
                         name="rhsA")
        rhsB = keep.tile([128, KG, 2, GRP * CHUNK], mmdt, tag="rhsB",
                         name="rhsB")
        rbuf = [rhsA, rhsB]
        dma_group(rhsA, 0)   # prologue: group 0 for the first iteration

        # ---- per-row 1/(||f||*TEMP) ----------------------------------------
        rn_s = keep.tile([128, RT], dt.float32, tag="rn")
        nOr_tmp = keep.tile([128, RT], dt.float32, tag="nOrt", name="nOr_tmp")
        for r in range(RT):
            ft = mk.tile([128, D], dt.float32, tag="ftile", bufs=1, name="ft")
            nc.gpsimd.dma_start(
                out=ft, in_=feat_d.ap()[r * 128:(r + 1) * 128, :])
            sq = mk.tile([128, D], dt.bfloat16, tag="sqscr", bufs=1, name="sq")
            n2 = fin.tile([128, 1], dt.float32, tag="n2")
            nc.vector.scalar_tensor_tensor(
                sq, in0=ft, scalar=0.0, in1=ft, op0=Alu.bypass,
                op1=Alu.mult, accum_out=n2)
            sn = fin.tile([128, 1], dt.float32, tag="sn")
            nc.scalar.activation(out=sn, in_=n2, func=Act.Sqrt, scale=TEMP * TEMP)
            nc.vector.reciprocal(out=rn_s[:, r:r + 1], in_=sn)
            nc.vector.tensor_scalar(nOr_tmp[:, r:r + 1], sn, -NEGBIG, None,
                                    op0=Alu.mult)
        # raw-scale mask constant: -NEGBIG * ||f|| * TEMP per row (applying
        # it to the un-normalized dot products keeps masked columns out of
        # the top-8 at raw scale)
        nOr_s = nOr_tmp
        # dummy Exp so the preamble exits with the exp/copy table loaded and
        # the loop body needs no per-iteration LoadActFuncSet
        dume = fin.tile([1, RT], dt.float32, tag="dume", name="dume")
        nc.scalar.activation(out=dume, in_=rn_s[0:1, :], func=Act.Exp)

        # ---- persistent per-row-tile state ---------------------------------
        U_all = keep.tile([128, RT, NCHUNK * 8], dt.bfloat16, tag="U",
                          name="U_all")
        U_s = [U_all[:, r, :] for r in range(RT)]
        NW = [max(1, len(windows[r])) for r in range(RT)]
        np_s = [keep.tile([128, NW[r]], dt.float32, tag=f"npos{r}", name=f"npos{r}")
                for r in range(RT)]
        sp_s = [keep.tile([128, NW[r]], dt.float32, tag=f"spos{r}", name=f"spos{r}")
                for r in range(RT)]
        pe_s = [keep.tile([128, NW[r]], dt.float32, tag=f"pexp{r}", name=f"pexp{r}")
                for r in range(RT)]
        for r in range(RT):
            nc.vector.memset(np_s[r], 0.0)
            nc.vector.memset(sp_s[r], 0.0)
            nc.vector.memset(pe_s[r], 0.0)
        stats_s = keep.tile([128, RT * 4], dt.float32, tag="stats",
                            name="stats")
        # rn for the host (un-scaling the shipped raw candidates)
        for r in range(RT):
            nc.vector.tensor_copy(out=stats_s[:, 4 * r + 3:4 * r + 4],
                                  in_=rn_s[:, r:r + 1])

        # ---- precompute window masks (DVE idle at kernel start) ------------
        wm = {}
        for r in range(RT):
            for w, pc in enumerate(windows[r]):
                if stage not in ("full", "noepi"):
                    continue
                wp = wpos[pc]
                ich = idx_s[:, wp * CHUNK:(wp + 1) * CHUNK]
                gsc = meta_s[:, r, 0:1]
                gec = meta_s[:, r, 1:2]
                csc = meta_s[:, r, 2:3]
                cec = meta_s[:, r, 3:4]
                a = mk.tile([128, CHUNK], dt.bfloat16, tag="a")
                nc.vector.tensor_scalar(a, ich, gsc, None, op0=Alu.is_ge)
                m1 = keep.tile([128, CHUNK], dt.bfloat16, tag=f"m1_{r}_{w}",
                               name=f"m1_{r}_{w}")
                nc.vector.scalar_tensor_tensor(
                    m1, in0=ich, scalar=gec, in1=a, op0=Alu.is_lt, op1=Alu.mult)
                bt = mk.tile([128, CHUNK], dt.bfloat16, tag="b")
                nc.vector.tensor_scalar(bt, ich, csc, None, op0=Alu.is_ge)
                m2 = mk.tile([128, CHUNK], dt.bfloat16, tag="m2")
                nc.vector.scalar_tensor_tensor(
                    m2, in0=ich, scalar=cec, in1=bt, op0=Alu.is_lt, op1=Alu.mult)
                mp = keep.tile([128, CHUNK], dt.bfloat16, tag=f"mp_{r}_{w}",
                               name=f"mp_{r}_{w}")
                nc.vector.scalar_tensor_tensor(
                    mp, in0=m2, scalar=-1.0, in1=m1, op0=Alu.mult, op1=Alu.add,
                    accum_out=np_s[r][:, w:w + 1])
                wm[(r, pc)] = (m1, mp)

        def stats_r(r):
            """Reduce the window accumulators for row-tile r into stats_s
            (npos, spos, pexp); top-50 selection + log run on host."""
            nc.vector.tensor_reduce(out=stats_s[:, 4 * r:4 * r + 1],
                                    in_=np_s[r],
                                    axis=mybir.AxisListType.X, op=Alu.add)
            nc.vector.tensor_reduce(out=stats_s[:, 4 * r + 1:4 * r + 2],
                                    in_=sp_s[r],
                                    axis=mybir.AxisListType.X, op=Alu.add)
            nc.vector.tensor_reduce(out=stats_s[:, 4 * r + 2:4 * r + 3],
                                    in_=pe_s[r],
                                    axis=mybir.AxisListType.X, op=Alu.add)

        # last group index containing any window chunk, per row-tile
        gw_last = [max((pc // GRP for pc in windows[r]), default=0)
                   for r in range(RT)]

        loop_cm = tc.For_i(0, loop_n, 1) if loop_n else contextlib.nullcontext()
        with loop_cm:
            # ---- main loop: 4-chunk groups, weight reused across 4 banks ---
            # Each (g, r, kg) block of 4 same-weight matmuls is pinned to an
            # increasing scheduler timestamp so the tile scheduler (whose PE
            # cost model runs DoubleRow ~2.3x too fast and would otherwise
            # chase freed PSUM banks down kg-chains) keeps the blocks whole;
            # the dedupe pass then drops 3 of every 4 LDWEIGHTS.
            for g in range(NGRP):
                rhs = rbuf[g % 2]
                # prefetch: next group into the other buffer; on the last
                # group, refill THIS iteration's group-0 buffer for the next
                # iteration (same DRAM contents every iteration)
                nxt = g + 1 if g + 1 < NGRP else 0
                dma_group(rbuf[nxt % 2], nxt)
                for r in range(RT):
                    pss = [psum.tile([128, CHUNK], dt.float32, tag="ps",
                                     name=f"ps{c}")
                           for c in range(GRP)]
                    for kg in range(KG):
                        for c in range(GRP):
                            nc.tensor.matmul(
                                pss[c],
                                lhsT=featT_s[:, kg, :, r * 128:(r + 1) * 128],
                                rhs=rhs[:, kg, :, c * CHUNK:(c + 1) * CHUNK],
                                start=(kg == 0),
                                stop=(kg == KG - 1),
                                perf_mode=mybir.MatmulPerfMode.DoubleRow,
                            )
                    if stage == "mmonly":
                        continue
                    for c in range(GRP):
                        pc = g * GRP + c
                        if pc in windows[r] and stage in ("full", "noepi"):
                            w = windows[r].index(pc)
                            m1, mp = wm[(r, pc)]
                            sims = evp.tile([128, CHUNK], dt.bfloat16,
                                            tag="sims")
                            nc.scalar.activation(
                                out=sims, in_=pss[c], func=Act.Copy,
                                scale=rn_s[:, r:r + 1])
                            scr = mk.tile([128, CHUNK], dt.bfloat16, tag="scr")
                            nc.vector.scalar_tensor_tensor(
                                scr, in0=sims, scalar=0.0, in1=mp,
                                op0=Alu.bypass, op1=Alu.mult,
                                accum_out=sp_s[r][:, w:w + 1])
                            ec = evp.tile([128, CHUNK], dt.bfloat16, tag="ec")
                            nc.scalar.activation(out=ec, in_=sims, func=Act.Exp)
                            scr2 = mk.tile([128, CHUNK], dt.bfloat16, tag="scr2")
                            nc.vector.scalar_tensor_tensor(
                                scr2, in0=ec, scalar=0.0, in1=mp,
                                op0=Alu.bypass, op1=Alu.mult,
                                accum_out=pe_s[r][:, w:w + 1])
                            # masked candidates at raw-dot scale for max8
                            negv = mk.tile([128, CHUNK], dt.bfloat16, tag="negv")
                            nc.vector.scalar_tensor_tensor(
                                negv, in0=m1, scalar=nOr_s[:, r:r + 1],
                                in1=pss[c], op0=Alu.mult, op1=Alu.add)
                            maxsrc = negv
                        else:
                            maxsrc = pss[c]
                        nc.vector.max(out=U_s[r][:, pc * 8:(pc + 1) * 8],
                                      in_=maxsrc)
                    if g == gw_last[r] and stage == "full":
                        stats_r(r)
                    if stage == "full":
                        if g == NGRP - 2:
                            # ship chunks [0, 3*GRP) once groups 0-2 are done
                            nc.sync.dma_start(
                                out=u_d.ap()[:, r * NCHUNK * 8:
                                             r * NCHUNK * 8 + (NGRP - 1) * GRP * 8],
                                in_=U_s[r][:, :(NGRP - 1) * GRP * 8])
                        elif g == NGRP - 1:
                            # ship the last group's candidates in one DMA
                            nc.sync.dma_start(
                                out=u_d.ap()[:, r * NCHUNK * 8 + (NGRP - 1) * GRP * 8:
                                             (r + 1) * NCHUNK * 8],
                                in_=U_s[r][:, (NGRP - 1) * GRP * 8:])
                if g == max(gw_last) and stage == "full":
                    nc.sync.dma_start(out=out_d.ap(), in_=stats_s)

            if stage == "full":
                pass  # stats DMA emitted inside the group loop
            else:
                nc.vector.memset(stats_s[:, 0:1], 0.0)
                nc.sync.dma_start(
                    out=out_d.ap()[0:1, 0:1], in_=stats_s[0:1, 0:1])
                nc.sync.dma_start(
                    out=u_d.ap()[0:1, 0:1], in_=U_all[0:1, 0, 0:1])

    nc.compile()
    return nc


_PROGRAM_CACHE = {}


def _get_program(windows):
    key = tuple(tuple(w) for w in windows)
    if key not in _PROGRAM_CACHE:
        _PROGRAM_CACHE[key] = _build_program(windows)
    return _PROGRAM_CACHE[key]


def kernel(features, proxy, targets, cams, pids, cids):
    from concourse.bass_utils import run_bass_kernel_spmd

    in_maps, windows = _prep(features, proxy, targets, cams, pids, cids)
    nc = _get_program(windows)
    res = run_bass_kernel_spmd(nc, in_maps, core_ids=list(range(NCORES)))
    return _finalize(res.results)
